# revision 8
# baseline (speedup 1.0000x reference)
"""Trainium2 Bass kernel for nn_CrossAttn (retrieval KNN cross-attention), v2.

Computation (see problem reference):
  1. KNN: for each pred point, top-8 nearest ref points by squared distance.
  2. Gather ref K/V features at those indices.
  3. Per-point softmax cross-attention over the 8 neighbors.
  4. Linear chain Wv/Wo/Wout; ref rows of the output are exactly zero.

Algebraic simplifications (host-side, exact):
  - Since attention weights sum to 1 and everything downstream of the
    attention-weighted sum is affine, gather RAW feat_v and apply the fused
    matrix  M = Wout @ Wo @ Wv  (and bias b = Wout @ (Wo @ bv + bo)) once per
    point:  out_pred = agg @ M^T + b.
  - First Nr output rows are zeros @ Wout^T == 0: never computed.
  - Ordering by -d2 equals ordering by (2*xp.xr - |xr|^2 - |xp|^2) per query
    row; the -|xp|^2 shift (5th contraction row) makes the PSUM value a true
    -d2 with small magnitude near the top, so an fp16 row preserves top-8
    ordering to ~fp32 fidelity (measured: +2/16384 flipped rows vs fp32).

Sharding: 8 cores x 2048 pred points (batch b = core//4, slice = core%4).
Only ref tables (xra, kv) are replicated per batch. No collectives.

Device pipeline per 128-query tile:
  PE fp32 distance matmuls -> ACT/Pool copy PSUM->SBUF as fp16 row ->
  DVE max/max_index (fp16, 2-byte fast mode) -> ONE batched indirect-DMA
  gather of 8 fp16 k|v rows per query -> DVE fused mul-reduce scores ->
  ACT exp / weighted products -> Pool/DVE add tree -> PE transpose + fp16
  fused output matmul -> DMA out (fp32).
"""

import os
import sys

if "/opt/trn_rl_repo" not in sys.path:
    sys.path.insert(0, "/opt/trn_rl_repo")

import numpy as np

B, NR, NP_, C, K = 2, 8192, 8192, 256, 8
NCORES = 8
SHARD = (B * NP_) // NCORES      # 2048 pred points per core
TILES = SHARD // 128             # 16
C2 = 2 * C                       # 512

# engine-balance knobs: of the 8 PSUM->SBUF row-copy chunks per tile, how
# many go to ACT (rest go to Pool); of the 7 add-tree adds, how many on Pool
N_COPY_ACT = 8
N_ADD_POOL = 7

# ---------------------------------------------------------------------------
# walrus workaround: this build rejects >1 sync-wait per instruction.
# Hoist extra waits onto standalone NoOps just before the instruction.
# ---------------------------------------------------------------------------
_uid = [0]


def _split_sync_waits(nc):
    import bass_rust
    import concourse.mybir as mybir

    n_split = 0
    for f in nc.m.functions:
        for bb in f.blocks:
            insts = bb.instructions
            if not any(
                i.sync_info is not None and len(i.sync_info.on_wait) > 1
                for i in insts
            ):
                continue
            new_list = []
            for inst in insts:
                si = inst.sync_info
                if si is not None and len(si.on_wait) > 1:
                    waits = list(si.on_wait)
                    for w in waits[:-1]:
                        _uid[0] += 1
                        nop = mybir.InstNoOp(
                            name=f"WSPLIT-{_uid[0]}", ins=[], outs=[])
                        nop.engine = inst.engine
                        nop.sync_info = bass_rust.SyncInfo(
                            on_wait=[w], on_update=[])
                        new_list.append(nop)
                    inst.sync_info = bass_rust.SyncInfo(
                        on_wait=[waits[-1]], on_update=list(si.on_update))
                    n_split += 1
                new_list.append(inst)
            insts.clear()
            insts.extend(new_list)
    return n_split


# ---------------------------------------------------------------------------
# device kernel
# ---------------------------------------------------------------------------
_NC_CACHE = None


def _build_nc():
    import concourse.bass as bass
    import concourse.mybir as mybir
    import concourse.tile as tile
    from concourse.masks import make_identity

    dt = mybir.dt
    f32 = dt.float32
    f16 = dt.float16
    nc = bass.Bass()

    xpa = nc.dram_tensor("xpa", [5, SHARD], f32, kind="ExternalInput")
    xra = nc.dram_tensor("xra", [5, NR], f32, kind="ExternalInput")
    q = nc.dram_tensor("q", [SHARD, C], f16, kind="ExternalInput")
    kv = nc.dram_tensor("kv", [NR, C2], f16, kind="ExternalInput")
    wt = nc.dram_tensor("wt", [C, C2], f16, kind="ExternalInput")
    bias = nc.dram_tensor("bias", [1, C2], f16, kind="ExternalInput")
    out = nc.dram_tensor("out", [SHARD, C2], f32, kind="ExternalOutput")

    with tile.TileContext(nc) as tc:
        with (
            tc.tile_pool(name="const", bufs=1) as constp,
            tc.tile_pool(name="row", bufs=2) as rowp,
            tc.tile_pool(name="io", bufs=2) as iop,
            tc.tile_pool(name="gat", bufs=2) as gatp,
            tc.tile_pool(name="small", bufs=2) as smallp,
            tc.tile_pool(name="work", bufs=2) as workp,
            tc.tile_pool(name="pd", bufs=2, space="PSUM") as pdp,
            tc.tile_pool(name="pt", bufs=2, space="PSUM") as ptp,
            tc.tile_pool(name="po", bufs=2, space="PSUM") as pop,
        ):
            # --- constants -------------------------------------------------
            xpa_sb = constp.tile([5, SHARD], f32, tag="xpa")
            nc.sync.dma_start(xpa_sb[:], xpa[:])
            xra_sb = constp.tile([5, NR], f32, tag="xra")
            nc.sync.dma_start(xra_sb[:], xra[:])
            wt0 = constp.tile([128, C2], f16, tag="wt0")
            nc.sync.dma_start(wt0[:], wt[0:128, :])
            wt1 = constp.tile([128, C2], f16, tag="wt1")
            nc.sync.dma_start(wt1[:], wt[128:256, :])
            bias_sb = constp.tile([1, C2], f16, tag="bias")
            nc.sync.dma_start(bias_sb[:], bias[:])
            ones_sb = constp.tile([1, 128], f16, tag="ones")
            nc.vector.memset(ones_sb[:], 1.0)
            ident = constp.tile([128, 128], f16, tag="ident")
            make_identity(nc, ident[:])

            for t in range(TILES):
                ts = slice(t * 128, (t + 1) * 128)
                # --- -d2 row in fp16: [128 queries, NR refs] ---------------
                row = rowp.tile([128, NR], f16, tag="row")
                for cch in range(8):
                    ps = pdp.tile([128, 1024], f32, tag="pd")
                    for h in range(2):
                        c0 = cch * 1024 + h * 512
                        nc.tensor.matmul(
                            ps[:, h * 512:(h + 1) * 512],
                            lhsT=xpa_sb[:, ts],
                            rhs=xra_sb[:, c0:c0 + 512],
                            start=True, stop=True,
                        )
                    dst = row[:, cch * 1024:(cch + 1) * 1024]
                    if cch < N_COPY_ACT:
                        nc.scalar.copy(dst, ps[:])
                    else:
                        nc.gpsimd.tensor_copy(dst, ps[:])

                # --- exact-fp16 top-8 (values + indices) -------------------
                vals = smallp.tile([128, K], f16, tag="vals")
                idx = smallp.tile([128, K], dt.uint32, tag="idx")
                nc.vector.max(out=vals[:], in_=row[:])
                nc.vector.max_index(out=idx[:], in_max=vals[:], in_values=row[:])

                # --- batched gather: 8 k|v rows (1KB fp16 each) per query --
                kvt = gatp.tile([128, K * C2], f16, tag="kvt")
                for j in range(K):
                    nc.gpsimd.indirect_dma_start(
                        out=kvt[:, j * C2:(j + 1) * C2],
                        out_offset=None, in_=kv[:],
                        in_offset=bass.IndirectOffsetOnAxis(
                            ap=idx[:, j:j + 1], axis=0),
                    )

                qt = iop.tile([128, C], f16, tag="q")
                nc.sync.dma_start(qt[:], q[ts, :])

                # --- scores: fused mul+reduce per neighbor -----------------
                scores = smallp.tile([128, K], f32, tag="scores")
                for j in range(K):
                    scr = workp.tile([128, C], f16, tag=f"scr{j}")
                    nc.vector.tensor_mul(scr[:], qt[:], kvt[:, j * C2:j * C2 + C])
                    nc.vector.reduce_sum(
                        scores[:, j:j + 1], scr[:], axis=mybir.AxisListType.X)
                # softmax weights: fold the 1/sqrt(C) scale into the exp
                e = smallp.tile([128, K], f32, tag="e")
                nc.scalar.activation(
                    e[:], scores[:], mybir.ActivationFunctionType.Exp,
                    scale=0.0625)
                ssum = smallp.tile([128, 1], f32, tag="ssum")
                nc.vector.reduce_sum(ssum[:], e[:], axis=mybir.AxisListType.X)
                rinv = smallp.tile([128, 1], f32, tag="rinv")
                nc.vector.reciprocal(rinv[:], ssum[:])

                # --- weighted sum of v (ACT products, Pool/DVE add tree) ---
                prods = []
                for j in range(K):
                    pj = workp.tile([128, C], f16, tag=f"prod{j}")
                    nc.scalar.activation(
                        pj[:], kvt[:, j * C2 + C:(j + 1) * C2],
                        mybir.ActivationFunctionType.Copy,
                        scale=e[:, j:j + 1],
                    )
                    prods.append(pj)
                lvl = prods
                li = 0
                n_pool_adds = 0
                while len(lvl) > 1:
                    nxt = []
                    for i in range(0, len(lvl), 2):
                        s = workp.tile([128, C], f16, tag=f"sum{li}_{i}")
                        if n_pool_adds < N_ADD_POOL:
                            nc.gpsimd.tensor_add(s[:], lvl[i][:], lvl[i + 1][:])
                            n_pool_adds += 1
                        else:
                            nc.vector.tensor_add(s[:], lvl[i][:], lvl[i + 1][:])
                        nxt.append(s)
                    lvl = nxt
                    li += 1
                agg = workp.tile([128, C], f16, tag="agg")
                nc.vector.tensor_scalar_mul(agg[:], lvl[0][:], rinv[:, 0:1])

                # --- out = agg @ M^T + b  (transpose agg, 3 fp16 matmuls) --
                aggT = []
                for h in range(2):
                    tp = ptp.tile([128, 128], f16, tag="pt")
                    nc.tensor.transpose(
                        tp[:], agg[:, h * 128:(h + 1) * 128], ident[:])
                    at = workp.tile([128, 128], f16, tag=f"aggT{h}")
                    nc.scalar.copy(at[:], tp[:])
                    aggT.append(at)
                po = pop.tile([128, C2], f32, tag="po")
                nc.tensor.matmul(po[:], lhsT=aggT[0][:], rhs=wt0[:],
                                 start=True, stop=False)
                nc.tensor.matmul(po[:], lhsT=aggT[1][:], rhs=wt1[:],
                                 start=False, stop=False)
                nc.tensor.matmul(po[:], lhsT=ones_sb[:], rhs=bias_sb[:],
                                 start=False, stop=True)
                out_sb = iop.tile([128, C2], f32, tag="outsb")
                nc.scalar.copy(out_sb[:], po[:])
                nc.sync.dma_start(out[ts, :], out_sb[:])

    _split_sync_waits(nc)
    return nc


def _get_nc():
    global _NC_CACHE
    if _NC_CACHE is None:
        _NC_CACHE = _build_nc()
    return _NC_CACHE


# exposed for test harness introspection (set after a traced run)
last_exec_time_ns = None
last_profile = None


def kernel(xyz_ref, xyz_pred, feat_k_ref, feat_q_pred, feat_v_ref,
           Wv, bv, Wo, bo, Wout):
    global last_exec_time_ns, last_profile
    from concourse.bass_utils import run_bass_kernel_spmd

    xyz_ref = np.asarray(xyz_ref, np.float32)
    xyz_pred = np.asarray(xyz_pred, np.float32)
    feat_k_ref = np.asarray(feat_k_ref, np.float32)
    feat_q_pred = np.asarray(feat_q_pred, np.float32)
    feat_v_ref = np.asarray(feat_v_ref, np.float32)
    Wv = np.asarray(Wv, np.float32)
    bv = np.asarray(bv, np.float32)
    Wo = np.asarray(Wo, np.float32)
    bo = np.asarray(bo, np.float32)
    Wout = np.asarray(Wout, np.float32)

    # fused output projection: out = agg @ (Wout@Wo@Wv)^T + Wout@(Wo@bv+bo)
    M = Wout @ (Wo @ Wv)                       # [512, 256]
    wt_np = np.ascontiguousarray(M.T).astype(np.float16)   # [256, 512]
    b_all = (Wout @ (Wo @ bv + bo)).reshape(1, C2).astype(np.float16)

    per_batch = {}
    for b in range(B):
        xr = xyz_ref[b]                                    # [NR, 3]
        xra = np.concatenate(
            [(2.0 * xr).T,
             (xr * xr).sum(1, keepdims=True).T,
             -np.ones((1, NR), np.float32)], axis=0)
        kv_np = np.concatenate(
            [feat_k_ref[b], feat_v_ref[b]], axis=1).astype(np.float16)
        per_batch[b] = (
            np.ascontiguousarray(xra, np.float32),
            np.ascontiguousarray(kv_np),
        )

    in_maps = []
    for core in range(NCORES):
        b, s = divmod(core, NCORES // B)
        sl = slice(s * SHARD, (s + 1) * SHARD)
        xp = xyz_pred[b, sl]                               # [SHARD, 3]
        xpa = np.concatenate(
            [xp.T,
             -np.ones((1, SHARD), np.float32),
             (xp * xp).sum(1, keepdims=True).T], axis=0)
        xra, kv_np = per_batch[b]
        in_maps.append({
            "xpa": np.ascontiguousarray(xpa, np.float32),
            "xra": xra,
            "q": np.ascontiguousarray(
                feat_q_pred[b, sl].astype(np.float16)),
            "kv": kv_np,
            "wt": wt_np,
            "bias": b_all,
        })

    trace = bool(int(os.environ.get("KERNEL_TRACE", "0")))
    res = run_bass_kernel_spmd(
        _get_nc(), in_maps, core_ids=list(range(NCORES)), trace=trace)
    last_exec_time_ns = res.exec_time_ns
    last_profile = res.profile_json

    outs = [r["out"] for r in res.results]                 # [2048, 512] x8
    pred = np.stack(
        [np.concatenate(outs[b * 4:(b + 1) * 4], axis=0) for b in range(B)])
    full = np.concatenate(
        [np.zeros((B, NR, C2), np.float32), pred], axis=1)
    return full


# revision 9
# speedup vs baseline: 1.0714x; 1.0714x over previous
"""Trainium2 Bass kernel for nn_CrossAttn (retrieval KNN cross-attention), v2.

Computation (see problem reference):
  1. KNN: for each pred point, top-8 nearest ref points by squared distance.
  2. Gather ref K/V features at those indices.
  3. Per-point softmax cross-attention over the 8 neighbors.
  4. Linear chain Wv/Wo/Wout; ref rows of the output are exactly zero.

Algebraic simplifications (host-side, exact):
  - Since attention weights sum to 1 and everything downstream of the
    attention-weighted sum is affine, gather RAW feat_v and apply the fused
    matrix  M = Wout @ Wo @ Wv  (and bias b = Wout @ (Wo @ bv + bo)) once per
    point:  out_pred = agg @ M^T + b.
  - First Nr output rows are zeros @ Wout^T == 0: never computed.
  - Ordering by -d2 equals ordering by (2*xp.xr - |xr|^2 - |xp|^2) per query
    row; the -|xp|^2 shift (5th contraction row) makes the PSUM value a true
    -d2 with small magnitude near the top, so an fp16 row preserves top-8
    ordering to ~fp32 fidelity (measured: +2/16384 flipped rows vs fp32).

Sharding: 8 cores x 2048 pred points (batch b = core//4, slice = core%4).
Only ref tables (xra, kv) are replicated per batch. No collectives.

Device pipeline per 128-query tile:
  PE fp32 distance matmuls -> ACT/Pool copy PSUM->SBUF as fp16 row ->
  DVE max/max_index (fp16, 2-byte fast mode) -> ONE batched indirect-DMA
  gather of 8 fp16 k|v rows per query -> DVE fused mul-reduce scores ->
  ACT exp / weighted products -> Pool/DVE add tree -> PE transpose + fp16
  fused output matmul -> DMA out (fp32).
"""

import os
import sys

if "/opt/trn_rl_repo" not in sys.path:
    sys.path.insert(0, "/opt/trn_rl_repo")

import numpy as np

B, NR, NP_, C, K = 2, 8192, 8192, 256, 8
NCORES = 8
SHARD = (B * NP_) // NCORES      # 2048 pred points per core
TILES = SHARD // 128             # 16
C2 = 2 * C                       # 512

# engine-balance knobs: of the 8 PSUM->SBUF row-copy chunks per tile, how
# many go to ACT (rest go to Pool); of the 7 add-tree adds, how many on Pool
N_COPY_ACT = 8
N_ADD_POOL = 7

# ---------------------------------------------------------------------------
# walrus workaround: this build rejects >1 sync-wait per instruction.
# Hoist extra waits onto standalone NoOps just before the instruction.
# ---------------------------------------------------------------------------
_uid = [0]


def _split_sync_waits(nc):
    import bass_rust
    import concourse.mybir as mybir

    n_split = 0
    for f in nc.m.functions:
        for bb in f.blocks:
            insts = bb.instructions
            if not any(
                i.sync_info is not None and len(i.sync_info.on_wait) > 1
                for i in insts
            ):
                continue
            new_list = []
            for inst in insts:
                si = inst.sync_info
                if si is not None and len(si.on_wait) > 1:
                    waits = list(si.on_wait)
                    for w in waits[:-1]:
                        _uid[0] += 1
                        nop = mybir.InstNoOp(
                            name=f"WSPLIT-{_uid[0]}", ins=[], outs=[])
                        nop.engine = inst.engine
                        nop.sync_info = bass_rust.SyncInfo(
                            on_wait=[w], on_update=[])
                        new_list.append(nop)
                    inst.sync_info = bass_rust.SyncInfo(
                        on_wait=[waits[-1]], on_update=list(si.on_update))
                    n_split += 1
                new_list.append(inst)
            insts.clear()
            insts.extend(new_list)
    return n_split


# ---------------------------------------------------------------------------
# device kernel
# ---------------------------------------------------------------------------
_NC_CACHE = None


def _build_nc():
    import concourse.bass as bass
    import concourse.mybir as mybir
    import concourse.tile as tile
    from concourse.masks import make_identity

    dt = mybir.dt
    f32 = dt.float32
    f16 = dt.float16
    nc = bass.Bass()

    xpa = nc.dram_tensor("xpa", [5, SHARD], f32, kind="ExternalInput")
    xra = nc.dram_tensor("xra", [5, NR], f32, kind="ExternalInput")
    q = nc.dram_tensor("q", [SHARD, C], f16, kind="ExternalInput")
    kv = nc.dram_tensor("kv", [NR, C2], f16, kind="ExternalInput")
    wt = nc.dram_tensor("wt", [C, C2], f16, kind="ExternalInput")
    bias = nc.dram_tensor("bias", [1, C2], f16, kind="ExternalInput")
    out = nc.dram_tensor("out", [SHARD, C2], f32, kind="ExternalOutput")

    with tile.TileContext(nc) as tc:
        with (
            tc.tile_pool(name="const", bufs=1) as constp,
            tc.tile_pool(name="row", bufs=2) as rowp,
            tc.tile_pool(name="io", bufs=2) as iop,
            tc.tile_pool(name="gat", bufs=2) as gatp,
            tc.tile_pool(name="small", bufs=2) as smallp,
            tc.tile_pool(name="work", bufs=2) as workp,
            tc.tile_pool(name="pd", bufs=2, space="PSUM") as pdp,
            tc.tile_pool(name="pt", bufs=2, space="PSUM") as ptp,
            tc.tile_pool(name="po", bufs=2, space="PSUM") as pop,
        ):
            # --- constants -------------------------------------------------
            xpa_sb = constp.tile([5, SHARD], f32, tag="xpa")
            nc.sync.dma_start(xpa_sb[:], xpa[:])
            xra_sb = constp.tile([5, NR], f32, tag="xra")
            nc.sync.dma_start(xra_sb[:], xra[:])
            wt0 = constp.tile([128, C2], f16, tag="wt0")
            nc.sync.dma_start(wt0[:], wt[0:128, :])
            wt1 = constp.tile([128, C2], f16, tag="wt1")
            nc.sync.dma_start(wt1[:], wt[128:256, :])
            bias_sb = constp.tile([1, C2], f16, tag="bias")
            nc.sync.dma_start(bias_sb[:], bias[:])
            ones_sb = constp.tile([1, 128], f16, tag="ones")
            nc.vector.memset(ones_sb[:], 1.0)
            ident = constp.tile([128, 128], f16, tag="ident")
            make_identity(nc, ident[:])

            for t in range(TILES):
                ts = slice(t * 128, (t + 1) * 128)
                # --- -d2 row in fp16: [128 queries, NR refs] ---------------
                row = rowp.tile([128, NR], f16, tag="row")
                for cch in range(8):
                    ps = pdp.tile([128, 1024], f32, tag="pd")
                    for h in range(2):
                        c0 = cch * 1024 + h * 512
                        nc.tensor.matmul(
                            ps[:, h * 512:(h + 1) * 512],
                            lhsT=xpa_sb[:, ts],
                            rhs=xra_sb[:, c0:c0 + 512],
                            start=True, stop=True,
                        )
                    dst = row[:, cch * 1024:(cch + 1) * 1024]
                    if cch < N_COPY_ACT:
                        nc.scalar.copy(dst, ps[:])
                    else:
                        nc.gpsimd.tensor_copy(dst, ps[:])

                # --- exact-fp16 top-8 (values + indices) -------------------
                vals = smallp.tile([128, K], f16, tag="vals")
                idx = smallp.tile([128, K], dt.uint32, tag="idx")
                nc.vector.max(out=vals[:], in_=row[:])
                nc.vector.max_index(out=idx[:], in_max=vals[:], in_values=row[:])

                # --- batched gather: 8 k|v rows (1KB fp16 each) per query --
                kvt = gatp.tile([128, K * C2], f16, tag="kvt")
                for j in range(K):
                    nc.gpsimd.indirect_dma_start(
                        out=kvt[:, j * C2:(j + 1) * C2],
                        out_offset=None, in_=kv[:],
                        in_offset=bass.IndirectOffsetOnAxis(
                            ap=idx[:, j:j + 1], axis=0),
                    )

                qt = iop.tile([128, C], f16, tag="q")
                nc.sync.dma_start(qt[:], q[ts, :])

                # --- scores: fused mul+reduce per neighbor -----------------
                scores = smallp.tile([128, K], f32, tag="scores")
                for j in range(K):
                    scr = workp.tile([128, C], f16, tag=f"scr{j}")
                    nc.vector.tensor_mul(scr[:], qt[:], kvt[:, j * C2:j * C2 + C])
                    nc.vector.reduce_sum(
                        scores[:, j:j + 1], scr[:], axis=mybir.AxisListType.X)
                # softmax weights: fold the 1/sqrt(C) scale into the exp
                e = smallp.tile([128, K], f32, tag="e")
                nc.scalar.activation(
                    e[:], scores[:], mybir.ActivationFunctionType.Exp,
                    scale=0.0625)
                ssum = smallp.tile([128, 1], f32, tag="ssum")
                nc.vector.reduce_sum(ssum[:], e[:], axis=mybir.AxisListType.X)
                rinv = smallp.tile([128, 1], f32, tag="rinv")
                nc.vector.reciprocal(rinv[:], ssum[:])

                # --- weighted sum of v (ACT products, Pool/DVE add tree) ---
                prods = []
                for j in range(K):
                    pj = workp.tile([128, C], f16, tag=f"prod{j}")
                    nc.scalar.activation(
                        pj[:], kvt[:, j * C2 + C:(j + 1) * C2],
                        mybir.ActivationFunctionType.Copy,
                        scale=e[:, j:j + 1],
                    )
                    prods.append(pj)
                lvl = prods
                li = 0
                n_pool_adds = 0
                while len(lvl) > 1:
                    nxt = []
                    for i in range(0, len(lvl), 2):
                        s = workp.tile([128, C], f16, tag=f"sum{li}_{i}")
                        if n_pool_adds < N_ADD_POOL:
                            nc.gpsimd.tensor_add(s[:], lvl[i][:], lvl[i + 1][:])
                            n_pool_adds += 1
                        else:
                            nc.vector.tensor_add(s[:], lvl[i][:], lvl[i + 1][:])
                        nxt.append(s)
                    lvl = nxt
                    li += 1
                agg = workp.tile([128, C], f16, tag="agg")
                nc.scalar.activation(
                    agg[:], lvl[0][:], mybir.ActivationFunctionType.Copy,
                    scale=rinv[:, 0:1])

                # --- out = agg @ M^T + b  (transpose agg, 3 fp16 matmuls) --
                aggT = []
                for h in range(2):
                    tp = ptp.tile([128, 128], f16, tag="pt")
                    nc.tensor.transpose(
                        tp[:], agg[:, h * 128:(h + 1) * 128], ident[:])
                    at = workp.tile([128, 128], f16, tag=f"aggT{h}")
                    nc.scalar.copy(at[:], tp[:])
                    aggT.append(at)
                po = pop.tile([128, C2], f32, tag="po")
                nc.tensor.matmul(po[:], lhsT=aggT[0][:], rhs=wt0[:],
                                 start=True, stop=False)
                nc.tensor.matmul(po[:], lhsT=aggT[1][:], rhs=wt1[:],
                                 start=False, stop=False)
                nc.tensor.matmul(po[:], lhsT=ones_sb[:], rhs=bias_sb[:],
                                 start=False, stop=True)
                out_sb = iop.tile([128, C2], f32, tag="outsb")
                nc.scalar.copy(out_sb[:], po[:])
                nc.sync.dma_start(out[ts, :], out_sb[:])

    _split_sync_waits(nc)
    return nc


def _get_nc():
    global _NC_CACHE
    if _NC_CACHE is None:
        _NC_CACHE = _build_nc()
    return _NC_CACHE


# exposed for test harness introspection (set after a traced run)
last_exec_time_ns = None
last_profile = None


def kernel(xyz_ref, xyz_pred, feat_k_ref, feat_q_pred, feat_v_ref,
           Wv, bv, Wo, bo, Wout):
    global last_exec_time_ns, last_profile
    from concourse.bass_utils import run_bass_kernel_spmd

    xyz_ref = np.asarray(xyz_ref, np.float32)
    xyz_pred = np.asarray(xyz_pred, np.float32)
    feat_k_ref = np.asarray(feat_k_ref, np.float32)
    feat_q_pred = np.asarray(feat_q_pred, np.float32)
    feat_v_ref = np.asarray(feat_v_ref, np.float32)
    Wv = np.asarray(Wv, np.float32)
    bv = np.asarray(bv, np.float32)
    Wo = np.asarray(Wo, np.float32)
    bo = np.asarray(bo, np.float32)
    Wout = np.asarray(Wout, np.float32)

    # fused output projection: out = agg @ (Wout@Wo@Wv)^T + Wout@(Wo@bv+bo)
    M = Wout @ (Wo @ Wv)                       # [512, 256]
    wt_np = np.ascontiguousarray(M.T).astype(np.float16)   # [256, 512]
    b_all = (Wout @ (Wo @ bv + bo)).reshape(1, C2).astype(np.float16)

    per_batch = {}
    for b in range(B):
        xr = xyz_ref[b]                                    # [NR, 3]
        xra = np.concatenate(
            [(2.0 * xr).T,
             (xr * xr).sum(1, keepdims=True).T,
             -np.ones((1, NR), np.float32)], axis=0)
        kv_np = np.concatenate(
            [feat_k_ref[b], feat_v_ref[b]], axis=1).astype(np.float16)
        per_batch[b] = (
            np.ascontiguousarray(xra, np.float32),
            np.ascontiguousarray(kv_np),
        )

    in_maps = []
    for core in range(NCORES):
        b, s = divmod(core, NCORES // B)
        sl = slice(s * SHARD, (s + 1) * SHARD)
        xp = xyz_pred[b, sl]                               # [SHARD, 3]
        xpa = np.concatenate(
            [xp.T,
             -np.ones((1, SHARD), np.float32),
             (xp * xp).sum(1, keepdims=True).T], axis=0)
        xra, kv_np = per_batch[b]
        in_maps.append({
            "xpa": np.ascontiguousarray(xpa, np.float32),
            "xra": xra,
            "q": np.ascontiguousarray(
                feat_q_pred[b, sl].astype(np.float16)),
            "kv": kv_np,
            "wt": wt_np,
            "bias": b_all,
        })

    trace = bool(int(os.environ.get("KERNEL_TRACE", "0")))
    res = run_bass_kernel_spmd(
        _get_nc(), in_maps, core_ids=list(range(NCORES)), trace=trace)
    last_exec_time_ns = res.exec_time_ns
    last_profile = res.profile_json

    outs = [r["out"] for r in res.results]                 # [2048, 512] x8
    pred = np.stack(
        [np.concatenate(outs[b * 4:(b + 1) * 4], axis=0) for b in range(B)])
    full = np.concatenate(
        [np.zeros((B, NR, C2), np.float32), pred], axis=1)
    return full
